# revision 12
# baseline (speedup 1.0000x reference)
"""Trainium2 Bass kernel for the gnn_message_passing reward environment.

reference:
    diff   = feature - next_feature                    # [N, D]
    neigh  = next_action @ diff                        # [N, D]
    impact = (neigh @ neigh.T) / D                     # [N, N]
    normed = row_l2_normalize(next_feature)            # [N, D]
    sim    = normed @ normed.T                         # [N, N]
    out    = persona_a * next_action * sim             # reward_sim
           - persona_b * edges                         # reward_cost
           + persona_g * impact                        # reward_impact
    (persona_x = persona_t @ x, per-row scalars)

Distribution: 1D row shard across 8 NeuronCores (512 rows each).
Input prep (dtype casts / transposes / the O(N*D) diff+normalize) happens
host-side in make_in_maps, all scaled into fp8e4m3 range.  On device each
core runs three row-sharded fp8 DoubleRow GEMMs with fp32 PSUM:
  GEMM1  neighT_own = diff.T @ A_own.T      (operands SBUF-resident)
  -> one fp8 AllGather of neighT (the only collective)
  GEMM2  sim shard  = ntl.T @ nt            (overlaps the AllGather)
  GEMM3  impact shard = neighT_own.T @ neighT_all
The elementwise reward combine is fused on DVE reading straight from PSUM
with per-row persona scalars; output is written bf16 and upcast host-side.
"""
import numpy as np
import ml_dtypes
from contextlib import ExitStack

import concourse.bass as bass
import concourse.tile as tile
from concourse import bacc, mybir
from concourse.bass_utils import run_bass_kernel_spmd

N = 4096          # graph nodes
D = 1024          # feature dim
NCORES = 8
R = N // NCORES   # 512 rows per core
RT = R // 128     # 4 row tiles per shard
DT = D // 128     # 8 d-tiles
NT = N // 128     # 32 n-tiles
NB = N // 512     # 8 output column blocks

F32 = mybir.dt.float32
BF16 = mybir.dt.bfloat16
F8 = mybir.dt.float8e4
MUL = mybir.AluOpType.mult
ADD = mybir.AluOpType.add
SUB = mybir.AluOpType.subtract
DR = mybir.MatmulPerfMode.DoubleRow

SD = 16.0         # host scale on diff        (fp8 carries 16*diff)
SN = 8.0          # scale on neigh            (fp8 carries 8*neigh)
ST = 16.0         # host scale on normed.T    (fp8 carries 16*normed.T)


def build(reps: int = 1, stage: int = 4, mock_cc: bool = False):
    nc = bacc.Bacc("TRN2", target_bir_lowering=False, debug=False,
                   num_devices=NCORES)

    difff = nc.dram_tensor("difff", [N, D], F8, kind="ExternalInput").ap()
    at8 = nc.dram_tensor("at8", [N, R], F8, kind="ExternalInput").ap()
    nt8 = nc.dram_tensor("nt8", [D, N], F8, kind="ExternalInput").ap()
    ntl8 = nc.dram_tensor("ntl8", [D, R], F8, kind="ExternalInput").ap()
    am8 = nc.dram_tensor("am8", [R, N], F8, kind="ExternalInput").ap()
    ed8 = nc.dram_tensor("ed8", [R, N], F8, kind="ExternalInput").ap()
    pvec = nc.dram_tensor("pvec", [128, 3 * RT], F32, kind="ExternalInput").ap()
    dum = nc.dram_tensor("dum", [1, 128], F8, kind="ExternalInput").ap()
    out = nc.dram_tensor("out", [R, N], BF16, kind="ExternalOutput").ap()

    rgroups = [list(range(NCORES))]

    def blk(ap):
        """[T*128, M] -> [128, T, M] partition-tiled view."""
        return ap.rearrange("(a p) m -> p a m", p=128)

    with tile.TileContext(nc) as tc, ExitStack() as ctx:
        const = ctx.enter_context(tc.tile_pool(name="const", bufs=1))
        res = ctx.enter_context(tc.tile_pool(name="res", bufs=1))
        stream = ctx.enter_context(tc.tile_pool(name="stream", bufs=1))
        outp_pool = ctx.enter_context(tc.tile_pool(name="outp", bufs=1))
        ps = ctx.enter_context(tc.tile_pool(name="ps", bufs=8, space="PSUM"))
        dram = ctx.enter_context(tc.tile_pool(name="dram", bufs=1, space="DRAM"))

        for rep in range(reps):
            # Tiny dummy collective issued immediately: absorbs the runtime's
            # first-collective global rendezvous (cross-core launch skew)
            # so the real AllGather below starts at its trigger instead of
            # waiting out the barrier.
            if rep == 0 and not mock_cc:
                dum_in = dram.tile([1, 128], F8, name="dum_in", tag="dumi")
                nc.sync.dma_start(dum_in[:], dum[:])
                dum_out = dram.tile([NCORES, 1, 128], F8, addr_space="Shared",
                                    name="dum_out", tag="dumo")
                nc.gpsimd.collective_compute(
                    "AllGather", mybir.AluOpType.bypass, ins=[dum_in.opt()],
                    outs=[dum_out.opt()], replica_groups=rgroups)

            pv_sb = const.tile([128, 3 * RT], F32, name=f"pv{rep}", tag="pv")
            nc.sync.dma_start(pv_sb[:], pvec[:])
            pa_sb = pv_sb[:, 0:RT]          # persona_alpha / (ST*ST)
            pbn_sb = pv_sb[:, RT:2 * RT]    # -persona_beta
            pgs_sb = pv_sb[:, 2 * RT:]      # persona_gamma / (SN*SN*D)

            # ---------------- resident operand loads ----------------
            # diff/at arrive in 8 contraction-chunks so GEMM1 starts as soon
            # as chunk 0 lands instead of after the full 6 MB load
            NC_CH = 8
            KPC = NT // NC_CH          # 4 n-tiles per chunk
            diff_ch, at_ch = [], []
            for c in range(NC_CH):
                ksl = slice(c * KPC * 128, (c + 1) * KPC * 128)
                dt_ = res.tile([128, KPC, D], F8, name=f"diff{rep}_{c}",
                               tag=f"diff{c}")
                nc.sync.dma_start(dt_[:], blk(difff[ksl, :]))
                diff_ch.append(dt_)
                at_ = res.tile([128, KPC, R], F8, name=f"at{rep}_{c}",
                               tag=f"at{c}")
                nc.sync.dma_start(at_[:], blk(at8[ksl, :]))
                at_ch.append(at_)
            nt_sb = res.tile([128, DT, N], F8, name=f"nt{rep}", tag="nt")
            nc.sync.dma_start(nt_sb[:], blk(nt8))
            ntl_sb = res.tile([128, DT, R], F8, name=f"ntl{rep}", tag="ntl")
            nc.sync.dma_start(ntl_sb[:], blk(ntl8))

            ag_in = dram.tile([D, R], F8, name=f"ag_in{rep}", tag="agi")
            ag_out = dram.tile([NCORES, D, R], F8, addr_space="Shared",
                               name=f"ag_out{rep}", tag="ago")

            # ---------------- GEMM1: neighT_own = diff.T @ A_own.T ----------
            # k-outer over all 8 psum banks: every bank completes at the end
            # of the contraction, which is when the AllGather needs them all
            ne_sb = res.tile([128, DT, R], F8, name=f"ne{rep}", tag="ne")
            g1b = [ps.tile([128, 512], F32, name=f"g1_{rep}_{d8}", tag="ps")
                   for d8 in range(DT)]
            KH = NT // 2
            for k2 in range(KH - 2):
                c, l = k2 // 2, k2 % 2
                for d8 in range(DT):
                    dsl = slice(d8 * 128, (d8 + 1) * 128)
                    nc.tensor.matmul(
                        g1b[d8][:], diff_ch[c][:, 2 * l:2 * l + 2, dsl],
                        at_ch[c][:, 2 * l:2 * l + 2, :],
                        start=(k2 == 0), stop=False, perf_mode=DR)
            # finish banks one at a time so the PSUM->fp8 copy and ag_in
            # write of bank d8 pipeline under bank d8+1's last matmuls
            for d8 in range(DT):
                dsl = slice(d8 * 128, (d8 + 1) * 128)
                for k2 in range(KH - 2, KH):
                    c, l = k2 // 2, k2 % 2
                    nc.tensor.matmul(
                        g1b[d8][:], diff_ch[c][:, 2 * l:2 * l + 2, dsl],
                        at_ch[c][:, 2 * l:2 * l + 2, :],
                        start=False, stop=(k2 == KH - 1), perf_mode=DR)
                # PSUM carries SD*neigh.T ; rescale to SN*neigh.T in fp8
                nc.scalar.mul(ne_sb[:, d8, :], g1b[d8][:], SN / SD)
                nc.sync.dma_start(ag_in[d8 * 128:(d8 + 1) * 128, :],
                                  ne_sb[:, d8, :])

            if mock_cc:
                nc.sync.dma_start(ag_out[0][:], ag_in[:])
            else:
                nc.gpsimd.collective_compute(
                    "AllGather", mybir.AluOpType.bypass, ins=[ag_in.opt()],
                    outs=[ag_out.opt()], replica_groups=rgroups)

            if stage <= 1:
                dbg = stream.tile([128, DT, R], BF16, name=f"dbg{rep}",
                                  tag="dbg")
                for d8 in range(DT):
                    nc.scalar.copy(dbg[:, d8, :], ne_sb[:, d8, :])
                    nc.sync.dma_start(out[0:128, d8 * 512:(d8 + 1) * 512],
                                      dbg[:, d8, :])
                continue

            # ---------------- GEMM2: sim + alpha*mask (overlaps AG) ---------
            outp = outp_pool.tile([128, RT, N], BF16, name=f"outp{rep}",
                                  tag="outp")
            for nb in range(NB):
                csl = slice(nb * 512, (nb + 1) * 512)
                am_t = stream.tile([128, RT, 512], F8, name=f"am{rep}_{nb}",
                                   tag="am", bufs=2)
                nc.sync.dma_start(am_t[:], blk(am8[:, csl]))
                for mt in range(RT):
                    msl = slice(mt * 128, (mt + 1) * 128)
                    sps = ps.tile([128, 512], F32, name=f"s{rep}_{nb}_{mt}",
                                  tag="ps")
                    for k2 in range(DT // 2):
                        nc.tensor.matmul(
                            sps[:], ntl_sb[:, 2 * k2:2 * k2 + 2, msl],
                            nt_sb[:, 2 * k2:2 * k2 + 2, csl],
                            start=(k2 == 0), stop=(k2 == DT // 2 - 1),
                            perf_mode=DR)
                    nc.vector.scalar_tensor_tensor(
                        outp[:, mt, csl], sps[:], pa_sb[:, mt:mt + 1],
                        am_t[:, mt, :], op0=MUL, op1=MUL)

            if stage <= 2:
                for mt in range(RT):
                    nc.sync.dma_start(out[mt * 128:(mt + 1) * 128, :],
                                      outp[:, mt, :])
                continue

            # ----- fold the edge cost into outp while the AllGather runs ----
            for nb in range(NB):
                csl = slice(nb * 512, (nb + 1) * 512)
                ed_t = stream.tile([128, RT, 512], F8, name=f"ed{rep}_{nb}",
                                   tag="ed", bufs=2)
                nc.sync.dma_start(ed_t[:], blk(ed8[:, csl]))
                for mt in range(RT):
                    nc.vector.scalar_tensor_tensor(
                        outp[:, mt, csl], ed_t[:, mt, :], pbn_sb[:, mt:mt + 1],
                        outp[:, mt, csl], op0=MUL, op1=ADD)

            # ---------------- GEMM3: impact + combine ----------------
            ner_sb = res.tile([128, DT, NCORES, 512], F8, name=f"ner{rep}",
                              tag="ner")
            for b in range(NCORES):
                nc.sync.dma_start(ner_sb[:, :, b, :], blk(ag_out[b]))
            for nb in range(NB):
                csl = slice(nb * 512, (nb + 1) * 512)
                o_blk = stream.tile([128, RT, 512], BF16,
                                    name=f"o{rep}_{nb}", tag="o_blk", bufs=2)
                for mt in range(RT):
                    msl = slice(mt * 128, (mt + 1) * 128)
                    ips = ps.tile([128, 512], F32, name=f"i{rep}_{nb}_{mt}",
                                  tag="ps")
                    for k2 in range(DT // 2):
                        nc.tensor.matmul(
                            ips[:], ne_sb[:, 2 * k2:2 * k2 + 2, msl],
                            ner_sb[:, 2 * k2:2 * k2 + 2, nb, :],
                            start=(k2 == 0), stop=(k2 == DT // 2 - 1),
                            perf_mode=DR)
                    nc.vector.scalar_tensor_tensor(
                        o_blk[:, mt, :], ips[:], pgs_sb[:, mt:mt + 1],
                        outp[:, mt, csl], op0=MUL, op1=ADD)
                    # per-mt output write: keeps the epilogue granule small
                    nc.sync.dma_start(out[msl, csl], o_blk[:, mt, :])

    nc.compile()
    return nc


_CACHE = {}


def _get_nc(reps=1, stage=4, mock_cc=False):
    key = (reps, stage, mock_cc)
    if key not in _CACHE:
        _CACHE[key] = build(reps, stage, mock_cc)
    return _CACHE[key]


def make_in_maps(feature, next_feature, next_action, edges, persona_t,
                 alpha, beta, gamma):
    F8NP = ml_dtypes.float8_e4m3
    feature = np.asarray(feature, np.float32)
    next_feature = np.asarray(next_feature, np.float32)
    next_action = np.asarray(next_action, np.float32)
    edges_np = np.asarray(edges, np.float32)
    persona_t = np.asarray(persona_t, np.float32)

    difff = ((feature - next_feature) * SD).astype(F8NP)
    norms = np.sqrt((next_feature * next_feature).sum(1, keepdims=True))
    normed_t = np.ascontiguousarray(
        ((next_feature / np.where(norms > 0, norms, 1.0)) * ST).T
    ).astype(F8NP)
    at_full = np.ascontiguousarray(next_action.T).astype(F8NP)
    am_full = next_action.astype(F8NP)
    ed_full = edges_np.astype(F8NP)

    pa = (persona_t @ np.asarray(alpha, np.float32)) / (ST * ST)
    pbn = -(persona_t @ np.asarray(beta, np.float32))
    pgs = (persona_t @ np.asarray(gamma, np.float32)) / (SN * SN * D)

    def pv(x, c):
        # [512] -> [128, RT] with pv[p, mt] = x[c*R + mt*128 + p]
        return np.ascontiguousarray(
            x[c * R:(c + 1) * R].reshape(RT, 128).T)

    in_maps = []
    for c in range(NCORES):
        rs = slice(c * R, (c + 1) * R)
        in_maps.append({
            "difff": difff,
            "at8": at_full[:, rs],
            "nt8": normed_t,
            "ntl8": np.ascontiguousarray(normed_t[:, rs]),
            "am8": am_full[rs],
            "ed8": ed_full[rs],
            "pvec": np.concatenate(
                [pv(pa, c), pv(pbn, c), pv(pgs, c)], axis=1
            ).astype(np.float32),
            "dum": np.zeros((1, 128), F8NP),
        })
    return in_maps


def kernel(feature, next_feature, next_action, edges, persona_t,
           alpha, beta, gamma):
    nc = _get_nc(1)
    in_maps = make_in_maps(feature, next_feature, next_action, edges,
                           persona_t, alpha, beta, gamma)
    res = run_bass_kernel_spmd(nc, in_maps, list(range(NCORES)))
    return np.concatenate(
        [res.results[c]["out"] for c in range(NCORES)], axis=0
    ).astype(np.float32)


# revision 14
# speedup vs baseline: 1.0775x; 1.0775x over previous
"""Trainium2 Bass kernel for the gnn_message_passing reward environment.

reference:
    diff   = feature - next_feature                    # [N, D]
    neigh  = next_action @ diff                        # [N, D]
    impact = (neigh @ neigh.T) / D                     # [N, N]
    normed = row_l2_normalize(next_feature)            # [N, D]
    sim    = normed @ normed.T                         # [N, N]
    out    = persona_a * next_action * sim             # reward_sim
           - persona_b * edges                         # reward_cost
           + persona_g * impact                        # reward_impact
    (persona_x = persona_t @ x, per-row scalars)

Distribution: 1D row shard across 8 NeuronCores (512 rows each).
Input prep (dtype casts / transposes / the O(N*D) diff+normalize) happens
host-side in make_in_maps, all scaled into fp8e4m3 range.  On device each
core runs three row-sharded fp8 DoubleRow GEMMs with fp32 PSUM:
  GEMM1  neighT_own = diff.T @ A_own.T      (operands SBUF-resident)
  -> one fp8 AllGather of neighT (the only collective)
  GEMM2  sim shard  = ntl.T @ nt            (overlaps the AllGather)
  GEMM3  impact shard = neighT_own.T @ neighT_all
The elementwise reward combine is fused on DVE reading straight from PSUM
with per-row persona scalars; output is written bf16 and upcast host-side.
"""
import numpy as np
import ml_dtypes
from contextlib import ExitStack

import concourse.bass as bass
import concourse.tile as tile
from concourse import bacc, mybir
from concourse.bass_utils import run_bass_kernel_spmd

N = 4096          # graph nodes
D = 1024          # feature dim
NCORES = 8
R = N // NCORES   # 512 rows per core
RT = R // 128     # 4 row tiles per shard
DT = D // 128     # 8 d-tiles
NT = N // 128     # 32 n-tiles
NB = N // 512     # 8 output column blocks

F32 = mybir.dt.float32
BF16 = mybir.dt.bfloat16
F8 = mybir.dt.float8e4
MUL = mybir.AluOpType.mult
ADD = mybir.AluOpType.add
SUB = mybir.AluOpType.subtract
DR = mybir.MatmulPerfMode.DoubleRow

SD = 16.0         # host scale on diff        (fp8 carries 16*diff)
SN = 8.0          # scale on neigh            (fp8 carries 8*neigh)
ST = 16.0         # host scale on normed.T    (fp8 carries 16*normed.T)


def build(reps: int = 1, stage: int = 4, mock_cc: bool = False):
    nc = bacc.Bacc("TRN2", target_bir_lowering=False, debug=False,
                   num_devices=NCORES)

    difff = nc.dram_tensor("difff", [N, D], F8, kind="ExternalInput").ap()
    at8 = nc.dram_tensor("at8", [N, R], F8, kind="ExternalInput").ap()
    nt8 = nc.dram_tensor("nt8", [D, N], F8, kind="ExternalInput").ap()
    ntl8 = nc.dram_tensor("ntl8", [D, R], F8, kind="ExternalInput").ap()
    am8 = nc.dram_tensor("am8", [R, N], F8, kind="ExternalInput").ap()
    ed8 = nc.dram_tensor("ed8", [R, N], F8, kind="ExternalInput").ap()
    pvec = nc.dram_tensor("pvec", [128, 3 * RT], F32, kind="ExternalInput").ap()
    dum = nc.dram_tensor("dum", [1, 128], F8, kind="ExternalInput").ap()
    out = nc.dram_tensor("out", [R, N], BF16, kind="ExternalOutput").ap()

    rgroups = [list(range(NCORES))]

    def blk(ap):
        """[T*128, M] -> [128, T, M] partition-tiled view."""
        return ap.rearrange("(a p) m -> p a m", p=128)

    with tile.TileContext(nc) as tc, ExitStack() as ctx:
        const = ctx.enter_context(tc.tile_pool(name="const", bufs=1))
        res = ctx.enter_context(tc.tile_pool(name="res", bufs=1))
        stream = ctx.enter_context(tc.tile_pool(name="stream", bufs=1))
        outp_pool = ctx.enter_context(tc.tile_pool(name="outp", bufs=1))
        ps = ctx.enter_context(tc.tile_pool(name="ps", bufs=8, space="PSUM"))
        dram = ctx.enter_context(tc.tile_pool(name="dram", bufs=1, space="DRAM"))

        for rep in range(reps):
            # Tiny dummy collective issued immediately: absorbs the runtime's
            # first-collective global rendezvous (cross-core launch skew)
            # so the real AllGather below starts at its trigger instead of
            # waiting out the barrier.
            if rep == 0 and not mock_cc:
                dum_in = dram.tile([1, 128], F8, name="dum_in", tag="dumi")
                nc.sync.dma_start(dum_in[:], dum[:])
                dum_out = dram.tile([NCORES, 1, 128], F8, addr_space="Shared",
                                    name="dum_out", tag="dumo")
                nc.gpsimd.collective_compute(
                    "AllGather", mybir.AluOpType.bypass, ins=[dum_in.opt()],
                    outs=[dum_out.opt()], replica_groups=rgroups)

            pv_sb = const.tile([128, 3 * RT], F32, name=f"pv{rep}", tag="pv")
            nc.sync.dma_start(pv_sb[:], pvec[:])
            pa_sb = pv_sb[:, 0:RT]          # persona_alpha / (ST*ST)
            pbn_sb = pv_sb[:, RT:2 * RT]    # -persona_beta
            pgs_sb = pv_sb[:, 2 * RT:]      # persona_gamma / (SN*SN*D)

            # ---------------- resident operand loads ----------------
            # diff/at arrive in 8 contraction-chunks so GEMM1 starts as soon
            # as chunk 0 lands instead of after the full 6 MB load
            NC_CH = 8
            KPC = NT // NC_CH          # 4 n-tiles per chunk
            diff_ch, at_ch = [], []
            for c in range(NC_CH):
                ksl = slice(c * KPC * 128, (c + 1) * KPC * 128)
                dt_ = res.tile([128, KPC, D], F8, name=f"diff{rep}_{c}",
                               tag=f"diff{c}")
                nc.sync.dma_start(dt_[:], blk(difff[ksl, :]))
                diff_ch.append(dt_)
                at_ = res.tile([128, KPC, R], F8, name=f"at{rep}_{c}",
                               tag=f"at{c}")
                nc.sync.dma_start(at_[:], blk(at8[ksl, :]))
                at_ch.append(at_)
            nt_sb = res.tile([128, DT, N], F8, name=f"nt{rep}", tag="nt")
            nc.sync.dma_start(nt_sb[:], blk(nt8))
            ntl_sb = res.tile([128, DT, R], F8, name=f"ntl{rep}", tag="ntl")
            nc.sync.dma_start(ntl_sb[:], blk(ntl8))

            ag_in = dram.tile([D, R], F8, name=f"ag_in{rep}", tag="agi")
            ag_out = dram.tile([NCORES, D, R], F8, addr_space="Shared",
                               name=f"ag_out{rep}", tag="ago")

            # ----- pre-fold the edge cost into outp while GEMM1 runs -----
            # (DVE is otherwise idle during GEMM1; gets it off the critical
            # path so GEMM3's combine never queues behind it)
            outp = outp_pool.tile([128, RT, N], BF16, name=f"outp{rep}",
                                  tag="outp")
            for nb in range(NB):
                csl = slice(nb * 512, (nb + 1) * 512)
                ed_t = stream.tile([128, RT, 512], F8, name=f"ed{rep}_{nb}",
                                   tag="ed", bufs=2)
                nc.sync.dma_start(ed_t[:], blk(ed8[:, csl]))
                for mt in range(RT):
                    nc.vector.tensor_scalar(
                        outp[:, mt, csl], ed_t[:, mt, :],
                        pbn_sb[:, mt:mt + 1], None, op0=MUL)

            # ---------------- GEMM1: neighT_own = diff.T @ A_own.T ----------
            # k-outer over all 8 psum banks: every bank completes at the end
            # of the contraction, which is when the AllGather needs them all
            ne_sb = res.tile([128, DT, R], F8, name=f"ne{rep}", tag="ne")
            g1b = [ps.tile([128, 512], F32, name=f"g1_{rep}_{d8}", tag="ps")
                   for d8 in range(DT)]
            KH = NT // 2
            for k2 in range(KH - 2):
                c, l = k2 // 2, k2 % 2
                for d8 in range(DT):
                    dsl = slice(d8 * 128, (d8 + 1) * 128)
                    nc.tensor.matmul(
                        g1b[d8][:], diff_ch[c][:, 2 * l:2 * l + 2, dsl],
                        at_ch[c][:, 2 * l:2 * l + 2, :],
                        start=(k2 == 0), stop=False, perf_mode=DR)
            # finish banks one at a time so the PSUM->fp8 copy and ag_in
            # write of bank d8 pipeline under bank d8+1's last matmuls
            for d8 in range(DT):
                dsl = slice(d8 * 128, (d8 + 1) * 128)
                for k2 in range(KH - 2, KH):
                    c, l = k2 // 2, k2 % 2
                    nc.tensor.matmul(
                        g1b[d8][:], diff_ch[c][:, 2 * l:2 * l + 2, dsl],
                        at_ch[c][:, 2 * l:2 * l + 2, :],
                        start=False, stop=(k2 == KH - 1), perf_mode=DR)
                # PSUM carries SD*neigh.T ; rescale to SN*neigh.T in fp8
                nc.scalar.mul(ne_sb[:, d8, :], g1b[d8][:], SN / SD)
                nc.sync.dma_start(ag_in[d8 * 128:(d8 + 1) * 128, :],
                                  ne_sb[:, d8, :])

            if mock_cc:
                nc.sync.dma_start(ag_out[0][:], ag_in[:])
            else:
                nc.gpsimd.collective_compute(
                    "AllGather", mybir.AluOpType.bypass, ins=[ag_in.opt()],
                    outs=[ag_out.opt()], replica_groups=rgroups)

            if stage <= 1:
                dbg = stream.tile([128, DT, R], BF16, name=f"dbg{rep}",
                                  tag="dbg")
                for d8 in range(DT):
                    nc.scalar.copy(dbg[:, d8, :], ne_sb[:, d8, :])
                    nc.sync.dma_start(out[0:128, d8 * 512:(d8 + 1) * 512],
                                      dbg[:, d8, :])
                continue

            # ---------------- GEMM2: sim + alpha*mask (overlaps AG) ---------
            for nb in range(NB):
                csl = slice(nb * 512, (nb + 1) * 512)
                am_t = stream.tile([128, RT, 512], F8, name=f"am{rep}_{nb}",
                                   tag="am", bufs=2)
                nc.sync.dma_start(am_t[:], blk(am8[:, csl]))
                for mt in range(RT):
                    msl = slice(mt * 128, (mt + 1) * 128)
                    sps = ps.tile([128, 512], F32, name=f"s{rep}_{nb}_{mt}",
                                  tag="ps")
                    for k2 in range(DT // 2):
                        nc.tensor.matmul(
                            sps[:], ntl_sb[:, 2 * k2:2 * k2 + 2, msl],
                            nt_sb[:, 2 * k2:2 * k2 + 2, csl],
                            start=(k2 == 0), stop=(k2 == DT // 2 - 1),
                            perf_mode=DR)
                    tmp_t = stream.tile([128, 512], BF16,
                                        name=f"t{rep}_{nb}_{mt}", tag="tmp",
                                        bufs=2)
                    nc.vector.scalar_tensor_tensor(
                        tmp_t[:], sps[:], pa_sb[:, mt:mt + 1],
                        am_t[:, mt, :], op0=MUL, op1=MUL)
                    nc.vector.tensor_tensor(
                        outp[:, mt, csl], outp[:, mt, csl], tmp_t[:], ADD)

            if stage <= 2:
                for mt in range(RT):
                    nc.sync.dma_start(out[mt * 128:(mt + 1) * 128, :],
                                      outp[:, mt, :])
                continue

            # ---------------- GEMM3: impact + combine ----------------
            ner_sb = res.tile([128, DT, NCORES, 512], F8, name=f"ner{rep}",
                              tag="ner")
            for b in range(NCORES):
                for kh in range(DT // 2):
                    nc.sync.dma_start(
                        ner_sb[:, 2 * kh:2 * kh + 2, b, :],
                        blk(ag_out[b][256 * kh:256 * (kh + 1), :]))
            for nb in range(NB):
                csl = slice(nb * 512, (nb + 1) * 512)
                o_blk = stream.tile([128, RT, 512], BF16,
                                    name=f"o{rep}_{nb}", tag="o_blk", bufs=2)
                for mt in range(RT):
                    msl = slice(mt * 128, (mt + 1) * 128)
                    ips = ps.tile([128, 512], F32, name=f"i{rep}_{nb}_{mt}",
                                  tag="ps")
                    for k2 in range(DT // 2):
                        nc.tensor.matmul(
                            ips[:], ne_sb[:, 2 * k2:2 * k2 + 2, msl],
                            ner_sb[:, 2 * k2:2 * k2 + 2, nb, :],
                            start=(k2 == 0), stop=(k2 == DT // 2 - 1),
                            perf_mode=DR)
                    nc.vector.scalar_tensor_tensor(
                        o_blk[:, mt, :], ips[:], pgs_sb[:, mt:mt + 1],
                        outp[:, mt, csl], op0=MUL, op1=ADD)
                    # per-mt output write: keeps the epilogue granule small
                    nc.sync.dma_start(out[msl, csl], o_blk[:, mt, :])

    nc.compile()
    return nc


_CACHE = {}


def _get_nc(reps=1, stage=4, mock_cc=False):
    key = (reps, stage, mock_cc)
    if key not in _CACHE:
        _CACHE[key] = build(reps, stage, mock_cc)
    return _CACHE[key]


def make_in_maps(feature, next_feature, next_action, edges, persona_t,
                 alpha, beta, gamma):
    F8NP = ml_dtypes.float8_e4m3
    feature = np.asarray(feature, np.float32)
    next_feature = np.asarray(next_feature, np.float32)
    next_action = np.asarray(next_action, np.float32)
    edges_np = np.asarray(edges, np.float32)
    persona_t = np.asarray(persona_t, np.float32)

    difff = ((feature - next_feature) * SD).astype(F8NP)
    norms = np.sqrt((next_feature * next_feature).sum(1, keepdims=True))
    normed_t = np.ascontiguousarray(
        ((next_feature / np.where(norms > 0, norms, 1.0)) * ST).T
    ).astype(F8NP)
    at_full = np.ascontiguousarray(next_action.T).astype(F8NP)
    am_full = next_action.astype(F8NP)
    ed_full = edges_np.astype(F8NP)

    pa = (persona_t @ np.asarray(alpha, np.float32)) / (ST * ST)
    pbn = -(persona_t @ np.asarray(beta, np.float32))
    pgs = (persona_t @ np.asarray(gamma, np.float32)) / (SN * SN * D)

    def pv(x, c):
        # [512] -> [128, RT] with pv[p, mt] = x[c*R + mt*128 + p]
        return np.ascontiguousarray(
            x[c * R:(c + 1) * R].reshape(RT, 128).T)

    in_maps = []
    for c in range(NCORES):
        rs = slice(c * R, (c + 1) * R)
        in_maps.append({
            "difff": difff,
            "at8": at_full[:, rs],
            "nt8": normed_t,
            "ntl8": np.ascontiguousarray(normed_t[:, rs]),
            "am8": am_full[rs],
            "ed8": ed_full[rs],
            "pvec": np.concatenate(
                [pv(pa, c), pv(pbn, c), pv(pgs, c)], axis=1
            ).astype(np.float32),
            "dum": np.zeros((1, 128), F8NP),
        })
    return in_maps


def kernel(feature, next_feature, next_action, edges, persona_t,
           alpha, beta, gamma):
    nc = _get_nc(1)
    in_maps = make_in_maps(feature, next_feature, next_action, edges,
                           persona_t, alpha, beta, gamma)
    res = run_bass_kernel_spmd(nc, in_maps, list(range(NCORES)))
    return np.concatenate(
        [res.results[c]["out"] for c in range(NCORES)], axis=0
    ).astype(np.float32)
